# revision 40
# baseline (speedup 1.0000x reference)
"""Trainium2 Bass kernel for nn_Delan_Sin (DeLaN-style batched tiny-MLP network).

Math folding (host side, exact algebra — same as the v1 kernel):
    z_m = M' @ sin(u1) + R_m @ qdd + bz_m          u1 = W1sin q + b1sin  (60)
    z_c = -A' @ sin^2(u1/2)·2 + R_c @ qd  + bz_c   (cos u = 1 - 2 sin^2(u/2))
    out = Wsig @ tanh([z_m; z_c]) + g_w2 @ sin(u_g) + b_out
with the 0.5 sigmoid-as-tanh scales folded into the z/Wsig weights.

v2 layout: everything is built around the engine floors of the CoreSim cost
model (matmul: out-cols/2.4GHz regardless of K; ACT: cols/1.2GHz + ~185ns
per instruction; DVE: cols/0.96GHz with 2x/4x bf16-SBUF modes; GPSIMD:
SBUF-only ops at cols/1.2GHz):

  - pair-packed phase-1 tiles (order S01|C01|S23|C23): S01 holds the sin-u
    rows of chunks 0 and 1 on partition halves, C01 the half-angle (u/2)
    rows.  ONE 2048-col Sin ACT covers all four banks per supergroup; the
    cos-representation squares are two 512-col DVE ops (120/128-row packed,
    2x bf16 mode).
  - u_g's sin is a degree-5 polynomial u*(PA*t^2+PB*t+PC), t=u*u, split
    across DVE+GPSIMD, keeping the ACT stream at sin(1892)+tanh(1038) =
    2930ns/supergroup against the PE floor of 14 matmuls = 2986ns.  Both
    engines run ~98% busy in steady state (period ~2980ns).
  - z per pair: 3 matmuls (Zs from sin rows, Zc from squared rows, Zx for
    qd/qdd/bias via the ones-row); tanh is one 1024-col ACT per supergroup.
  - final: GW2-quad writes a dense [46, 512] psum tile (rows OROW[a]+o,
    32-aligned halves), Wsig pair matmuls accumulate at tile_position 0/32.
    Output DMA is a dense [46, 512] f32 tile (4.5x less traffic than v1).
  - 3-stage software-pipelined loop; PSUM is exactly 8 banks: U4(4) + G(1)
    + Z(2) + OP(1).  Drain specials: the last supergroup's z reuses the
    dead U4 banks (dodges the Z-tile WAR on the previous tanh), its u_g
    pipeline is hoisted two iterations early, and its tanh is split per
    pair.  Fill specials: phase-1 stationaries DMA first, z/final
    stationaries right after xs0.
"""

import numpy as np

DOF = 7
HID = 30
B = 262144
N_CORES = 8
BC = B // N_CORES          # 32768 rows per core
CH = 512                   # chunk = matmul moving dim (one PSUM bank)
NSG = 16                   # supergroups per core (4 chunks each)

OROW = (0, 7, 32, 39)      # dense out row base per chunk (32-aligned halves)

# cstb column blocks (bf16 [128, C2])
CB_ZS = 0        # Zs pair blob      [128, 128]
CB_ZC = 128      # Zc pair blob      [128, 128]
CB_ZX0 = 256     # Zx chunks 0,1     [128, 128]
CB_ZX1 = 384     # Zx chunks 2,3     [128, 128]
CB_WS = 512      # Wsig pair blob    [128, 16] (14 used)
CB_GW = 528      # GW2 quad blob     [128, 48] (46 used)
CB_S01 = 576     # phase-1 sin-u, chunks 0,1   [128, 128]
CB_S23 = 704
CB_C01 = 832     # phase-1 u/2 rows (half-angle)
CB_C23 = 960
CB_G = 1088      # phase-1 u_g quad  [128, 128] (120 used)
C2 = 1216

# degree-5 sin poly on [-2.2, 2.2]: sin(u) ~ u*(PA*t^2 + PB*t + PC), t = u^2
PA = 0.006794867776426575
PB = -0.16306606860468015
PC = 0.9978675740619841

_BUILD_CACHE = {}


def _emit_poly(nc, sgp, sgt, u, s, BF16, MULT, ADD, suffix=""):
    """u_g sin poly: s_g = u*(PA*t^2 + PB*t + PC), t = u*u.
    Pool does the muls/FMAs that only touch SBUF; DVE the rest."""
    t2 = sgp.tile([128, CH], BF16, tag="t2" + suffix)
    nc.gpsimd.tensor_mul(t2[:], u[:], u[:])
    f1 = sgp.tile([128, CH], BF16, tag="f1" + suffix)
    nc.gpsimd.tensor_scalar(f1[:], t2[:], PA, PB, MULT, ADD)
    g2 = sgp.tile([128, CH], BF16, tag="g2" + suffix)
    nc.vector.tensor_mul(g2[:], f1[:], t2[:])
    f2 = sgp.tile([128, CH], BF16, tag="f2" + suffix)
    nc.gpsimd.tensor_scalar(f2[:], g2[:], 1.0, PC, MULT, ADD)
    sg = sgp.tile([128, CH], BF16, tag="scg" + suffix)
    nc.vector.tensor_mul(sg[:], u[:], f2[:])
    sgt[s] = sg


def _f(a):
    return np.asarray(a, dtype=np.float64)


def fold_weights(inp):
    """Fold all 5 MLPs into the small dense matrices used on device (float64
    folding, cast at the end). Field layout kept compatible with test.py's
    host_folded_forward."""
    ld_w1, ld_b1 = _f(inp["ld_w1"]), _f(inp["ld_b1"])
    ld_w2, ld_b2 = _f(inp["ld_w2"]), _f(inp["ld_b2"])
    lo_w1, lo_b1 = _f(inp["lo_w1"]), _f(inp["lo_b1"])
    lo_w2, lo_b2 = _f(inp["lo_w2"]), _f(inp["lo_b2"])
    g_w1, g_b1 = _f(inp["g_w1"]), _f(inp["g_b1"])
    g_w2, g_b2 = _f(inp["g_w2"]), _f(inp["g_b2"])
    m_w1, m_b1 = _f(inp["m_w1"]), _f(inp["m_b1"])
    m_w2, m_b2 = _f(inp["m_w2"]), _f(inp["m_b2"])
    c_w1, c_b1 = _f(inp["c_w1"]), _f(inp["c_b1"])
    c_w2, c_b2 = _f(inp["c_w2"]), _f(inp["c_b2"])

    # m-net first layer folded through h_l
    M_ld = m_w1[:, :DOF] @ ld_w2                      # [30, 30]
    M_lo = m_w1[:, DOF : 4 * DOF] @ lo_w2             # [30, 30]
    R_m = m_w1[:, 4 * DOF :]                          # [30, 7]
    bz_m = m_b1 + m_w1[:, :DOF] @ ld_b2 + m_w1[:, DOF : 4 * DOF] @ lo_b2

    # c-net first layer folded through the jacobian contraction
    cw = c_w1[:, : 28 * DOF].reshape(HID, 28, DOF)    # [j, i, d]
    A_ld = np.einsum("jid,ih,hd->jh", cw[:, :DOF, :], ld_w2, ld_w1)
    A_lo = np.einsum("jid,ih,hd->jh", cw[:, DOF:, :], lo_w2, lo_w1)
    R_c = c_w1[:, 28 * DOF :]                         # [30, 7]

    # padded-row first layer (kept for test.py's host_folded_forward)
    R_SLD, R_SLO, R_CLD, R_CLO = 0, 30, 64, 96
    W1dup = np.zeros((128, DOF))
    b1dup = np.zeros(128)
    W1dup[R_SLD : R_SLD + HID] = ld_w1
    W1dup[R_SLO : R_SLO + HID] = lo_w1
    W1dup[R_CLD : R_CLD + HID] = ld_w1
    W1dup[R_CLO : R_CLO + HID] = lo_w1
    b1dup[R_SLD : R_SLD + HID] = ld_b1
    b1dup[R_SLO : R_SLO + HID] = lo_b1
    b1dup[R_CLD : R_CLD + HID] = ld_b1 / 2
    b1dup[R_CLO : R_CLO + HID] = lo_b1 / 2

    G1 = np.zeros((64, 128))
    G1[0:HID, R_SLD : R_SLD + HID] = M_ld
    G1[0:HID, R_SLO : R_SLO + HID] = M_lo
    G1[HID : 2 * HID, R_CLD : R_CLD + HID] = -2.0 * A_ld
    G1[HID : 2 * HID, R_CLO : R_CLO + HID] = -2.0 * A_lo
    G2 = np.zeros((64, 2 * DOF))
    G2[0:HID, DOF:] = R_m
    G2[HID : 2 * HID, 0:DOF] = R_c
    b_z = np.zeros(64)
    b_z[0:HID] = bz_m
    b_z[HID : 2 * HID] = c_b1 + A_ld.sum(axis=1) + A_lo.sum(axis=1)
    W1dup[64:128] /= 2.0

    Wsig = np.concatenate([m_w2, c_w2], axis=1)       # [7, 60]
    b_out = m_b2 + c_b2 + g_b2
    # sigmoid(z) = 0.5 + 0.5*tanh(z/2)
    G1 *= 0.5
    G2 *= 0.5
    b_z *= 0.5
    b_out = b_out + 0.5 * Wsig.sum(axis=1)
    Wsig = 0.5 * Wsig

    return dict(
        W1dup=W1dup, b1dup=b1dup, g_w1=g_w1, g_b1=g_b1,
        G1=G1, G2=G2, b_z=b_z, Wsig=Wsig, g_w2=g_w2, b_out=b_out,
    )


def build_const_blobs(fw):
    """v2 stationary blob [128, C2] bf16."""
    import ml_dtypes

    cstb = np.zeros((128, C2), dtype=np.float32)
    G1 = fw["G1"]            # [64, 128]: rows 0:30 = zm from sin-rows (cols
    #                          0:60 of the old sc layout), rows 30:60 = zc
    #                          from sq-rows (old cols 64:124), 0.5-folded
    Mp = G1[0:HID, 0:60]                 # zm <- sin rows   [30, 60]
    A2 = G1[HID : 2 * HID, 64:124]       # zc <- sq rows    [30, 60]
    G2 = fw["G2"]            # [64, 14]: cols qd(0:7) qdd(7:14), 0.5-folded
    b_z = fw["b_z"]          # [64]
    W1dup = fw["W1dup"]      # [128, 7] (rows 0:60 sin-u, 64:124 u/2)
    b1dup = fw["b1dup"]
    W1sin, b1sin = W1dup[0:60], b1dup[0:60]
    W1half, b1half = W1dup[64:124], b1dup[64:124]
    Wsig = fw["Wsig"]        # [7, 60]
    g_w1, g_b1 = fw["g_w1"], fw["g_b1"]
    g_w2 = fw["g_w2"]        # [7, 30]

    for e in range(2):
        r0, z0 = 64 * e, 64 * e
        # Zs: sc-S rows (sin values of chunk e) -> zm rows of chunk e
        cstb[r0 : r0 + 60, CB_ZS + z0 : CB_ZS + z0 + HID] = Mp.T
        # Zc: sc-C rows (sin^2 of chunk e) -> zc rows
        cstb[r0 : r0 + 60, CB_ZC + z0 + HID : CB_ZC + z0 + 2 * HID] = A2.T
        # Wsig: tanh(z) rows of chunk e -> out rows 7e+o
        cstb[r0 : r0 + 60, CB_WS + 7 * e : CB_WS + 7 * e + DOF] = Wsig.T
    for a in range(4):
        cb_zx = CB_ZX0 if a < 2 else CB_ZX1
        e = a % 2
        xr = 32 * a
        z0 = 64 * e
        # qd -> zc, qdd -> zm, ones -> b_z
        cstb[xr + DOF : xr + 2 * DOF, cb_zx + z0 + HID : cb_zx + z0 + 2 * HID] = (
            G2[HID : 2 * HID, 0:DOF].T
        )
        cstb[xr + 2 * DOF : xr + 3 * DOF, cb_zx + z0 : cb_zx + z0 + HID] = (
            G2[0:HID, DOF:].T
        )
        cstb[xr + 21, cb_zx + z0 : cb_zx + z0 + 2 * HID] = b_z[0 : 2 * HID]
        # GW2 quad: sc-G rows 30a.. -> out rows OROW[a]+o
        cstb[30 * a : 30 * a + HID, CB_GW + OROW[a] : CB_GW + OROW[a] + DOF] = g_w2.T
        # phase-1 u_g: x rows of chunk a -> G rows 30a..
        cstb[xr : xr + DOF, CB_G + 30 * a : CB_G + 30 * a + HID] = g_w1.T
        cstb[xr + 21, CB_G + 30 * a : CB_G + 30 * a + HID] = g_b1
    for pair, (cb_s, cb_c) in enumerate(((CB_S01, CB_C01), (CB_S23, CB_C23))):
        for e in range(2):
            a = 2 * pair + e
            xr, r0 = 32 * a, 64 * e
            cstb[xr : xr + DOF, cb_s + r0 : cb_s + r0 + 60] = W1sin.T
            cstb[xr + 21, cb_s + r0 : cb_s + r0 + 60] = b1sin
            cstb[xr : xr + DOF, cb_c + r0 : cb_c + r0 + 60] = W1half.T
            cstb[xr + 21, cb_c + r0 : cb_c + r0 + 60] = b1half
    return cstb.astype(ml_dtypes.bfloat16)


def pack_x_core(x_core):
    """[32768, 21] f32 -> [128, 8192] bf16; chunk 4t+a row f at [32a+f, 512t:].

    Rows 32a+21 are the constant-ones bias rows; 32a+22..32a+31 stay zero
    (K=128 matmuls contract the full partition range)."""
    import ml_dtypes

    xc = np.ascontiguousarray(x_core, dtype=np.float32).reshape(NSG, 4, CH, 3 * DOF)
    xp = np.zeros((4, 32, NSG, CH), dtype=np.float32)
    xp[:, : 3 * DOF] = xc.transpose(1, 3, 0, 2)
    xp[:, 21] = 1.0
    return np.ascontiguousarray(
        xp.reshape(128, NSG * CH).astype(ml_dtypes.bfloat16)
    )


def unpack_out_core(oh, b_out):
    """[46, 8192] bf16 -> [32768, 7]; chunk 4t+a output o at row OROW[a]+o,
    col 512t+j."""
    oh = np.asarray(oh, dtype=np.float32)
    o = np.empty((4, DOF, NSG, CH), dtype=np.float32)
    for a in range(4):
        o[a] = oh[OROW[a] : OROW[a] + DOF].reshape(DOF, NSG, CH)
    o = o.transpose(2, 0, 3, 1).reshape(BC, DOF)     # [t, a, j, o]
    return o + b_out[None, :].astype(np.float32)


def _build_bass():
    """Build the (input-independent) Bass program once."""
    if "nc" in _BUILD_CACHE:
        return _BUILD_CACHE["nc"]

    import concourse.bacc as bacc
    import concourse.tile as tile
    from concourse import mybir

    F32 = mybir.dt.float32
    BF16 = mybir.dt.bfloat16
    SIN = mybir.ActivationFunctionType.Sin
    TANH = mybir.ActivationFunctionType.Tanh
    SUB = mybir.AluOpType.subtract
    MULT = mybir.AluOpType.mult
    ADD = mybir.AluOpType.add

    # Co-locate Sin and Tanh in one ACT table set so the kernel does a single
    # table load (strip them from every set except silu_and_others).
    if not getattr(bacc, "_delan_act_tables_patched", False):
        _orig_gat = bacc.get_activation_tables

        def _gat(arch):
            t = _orig_gat(arch)
            out = {}
            for name, funcs in t.items():
                if name != "silu_and_others":
                    funcs = funcs - {SIN, TANH}
                out[name] = funcs
            return out

        bacc.get_activation_tables = _gat
        bacc._delan_act_tables_patched = True

    nc = bacc.Bacc("TRN2", target_bir_lowering=False, debug=False)

    xt_d = nc.dram_tensor("xt", [128, NSG * CH], BF16, kind="ExternalInput").ap()
    cstb_d = nc.dram_tensor("cstb", [128, C2], BF16, kind="ExternalInput").ap()
    out_d = nc.dram_tensor("out", [46, NSG * CH], BF16, kind="ExternalOutput").ap()

    with tile.TileContext(nc) as tc:
        with (
            tc.tile_pool(name="consts", bufs=1) as consts,
            tc.tile_pool(name="xp", bufs=NSG) as xp,
            tc.tile_pool(name="scp", bufs=2) as scp,
            tc.tile_pool(name="sgp", bufs=2) as sgp,
            tc.tile_pool(name="sigp", bufs=2) as sigp,
            tc.tile_pool(name="oup", bufs=2) as oup,
            tc.tile_pool(name="ps_u", bufs=1, space="PSUM") as ps_u,
            tc.tile_pool(name="ps_g", bufs=1, space="PSUM") as ps_g,
            tc.tile_pool(name="ps_z", bufs=1, space="PSUM") as ps_z,
            tc.tile_pool(name="ps_o", bufs=1, space="PSUM") as ps_o,
        ):
            cstb = consts.tile([128, C2], BF16)
            # phase-1 stationaries first so the first matmul starts sooner;
            # the z/final stationaries right after xs0/xs1 so the z matmuls
            # of the first supergroup aren't gated on the whole xs stream.
            nc.sync.dma_start(out=cstb[:, CB_S01:C2], in_=cstb_d[:, CB_S01:C2])
            xs = []
            for t in range(NSG):
                x = xp.tile([128, CH], BF16, tag="xs")
                nc.sync.dma_start(out=x[:], in_=xt_d[:, t * CH : (t + 1) * CH])
                xs.append(x)
                if t == 0:
                    nc.sync.dma_start(out=cstb[:, 0:CB_S01], in_=cstb_d[:, 0:CB_S01])

            # pipeline state (stage outputs), rotated by the tile pools
            sc = [None, None]       # sin ACT output [128, 2048] bf16
            ug = [None, None]       # u_g copied to SBUF bf16
            sgt = {}                # poly scG tiles, keyed by supergroup
            sig = [None, None]      # tanh output [128, 1024] bf16

            # U4 / sc4 bank order: S01 | C01 | S23 | C23
            for it in range(NSG + 2):
                # squares of the C halves for s=it-1: FIRST in DVE program
                # order so they complete before the Zc matmuls need them.
                # (the last supergroup's squares are emitted at the end of
                # iteration NSG-1 instead)
                if 1 <= it <= NSG - 1:
                    s4p = sc[(it - 1) % 2]
                    for q in (1, 3):
                        nc.vector.tensor_mul(
                            s4p[:, q * CH : (q + 1) * CH],
                            s4p[:, q * CH : (q + 1) * CH],
                            s4p[:, q * CH : (q + 1) * CH],
                        )

                # ---------------- stage A: phase-1 (t = it) ----------------
                if it < NSG:
                    t = it
                    x = xs[t]
                    U = ps_u.tile([128, 4 * CH], F32, tag="u4")
                    s4 = scp.tile([128, 4 * CH], BF16, tag="sc")
                    for k, cb in enumerate((CB_S01, CB_C01, CB_S23, CB_C23)):
                        nc.tensor.matmul(
                            U[:, k * CH : (k + 1) * CH],
                            cstb[:, cb : cb + 128],
                            x[:],
                            start=True, stop=True,
                        )
                    nc.scalar.activation(out=s4[:], in_=U[:], func=SIN)
                    sc[t % 2] = s4
                    if t < NSG - 1:
                        G = ps_g.tile([128, CH], F32, tag="g")
                        nc.tensor.matmul(
                            G[:], cstb[:, CB_G : CB_G + 128], x[:],
                            start=True, stop=True,
                        )
                        u = sgp.tile([128, CH], BF16, tag="ug")
                        nc.vector.tensor_copy(u[:], G[:])
                        ug[t % 2] = u

                # stage C head (r = it-2): GW2q ahead of the z matmuls —
                # delays the z-group one PE slot so its wait on tanh(it-2)'s
                # Z-tile read has margin, and opens the OP accumulation group
                if it >= 2:
                    r = it - 2
                    sgr = sgt.pop(r)
                    OP = ps_o.tile([128, CH], F32, tag="op")
                    nc.tensor.matmul(
                        OP[0:46, :], cstb[:, CB_GW : CB_GW + 46], sgr[:],
                        start=True, stop=False, skip_group_check=True,
                    )

                # ---------------- stage B: z + tanh + poly (s = it-1) ------
                if 1 <= it <= NSG:
                    s = it - 1
                    s4, x = sc[s % 2], xs[s]
                    if s < NSG - 1:
                        _emit_poly(nc, sgp, sgt, ug[s % 2], s, BF16, MULT, ADD)

                    last = s == NSG - 1
                    if last:
                        # drain: the last supergroup's z lives in the (dead)
                        # U4 banks, so it need not wait for tanh(14)'s read
                        # of the shared Z tile
                        Z4 = ps_u.tile([128, 4 * CH], F32, tag="u4")
                        Z = Z4[:, 0 : 2 * CH]
                    else:
                        Z = ps_z.tile([128, 2 * CH], F32, tag="z")
                    sg2 = sigp.tile([128, 2 * CH], BF16, tag="sig")
                    for p in range(2):
                        zc = Z[:, p * CH : (p + 1) * CH]
                        # Zx starts the group: it only needs xs, so it can
                        # dispatch before the sin ACT completes
                        nc.tensor.matmul(
                            zc, cstb[:, CB_ZX0 + 128 * p : CB_ZX0 + 128 * (p + 1)],
                            x[:],
                            start=True, stop=False, skip_group_check=True,
                        )
                        nc.tensor.matmul(
                            zc, cstb[:, CB_ZS : CB_ZS + 128],
                            s4[:, 2 * p * CH : (2 * p + 1) * CH],
                            start=False, stop=False, skip_group_check=True,
                        )
                        zc_src = (
                            sq_last[:, p * CH : (p + 1) * CH] if last
                            else s4[:, (2 * p + 1) * CH : (2 * p + 2) * CH]
                        )
                        nc.tensor.matmul(
                            zc, cstb[:, CB_ZC : CB_ZC + 128],
                            zc_src,
                            start=False, stop=True, skip_group_check=True,
                        )
                    if last:
                        # drain: per-pair tanh (emitted after all six z
                        # matmuls) so downstream stages start earlier
                        for p in range(2):
                            nc.scalar.activation(
                                out=sg2[:, p * CH : (p + 1) * CH],
                                in_=Z[:, p * CH : (p + 1) * CH], func=TANH,
                            )
                    else:
                        nc.scalar.activation(out=sg2[:], in_=Z[:], func=TANH)
                    sig[s % 2] = sg2

                # last supergroup's squares: into a separate tile so the
                # Zs matmuls of the drain don't pick up a false write-after
                # dependency on sc4
                if it == NSG - 1:
                    s4p = sc[it % 2]
                    sqc = scp.tile([128, 2 * CH], BF16, tag="sqc")
                    for qi, q in enumerate((1, 3)):
                        nc.vector.tensor_mul(
                            sqc[:, qi * CH : (qi + 1) * CH],
                            s4p[:, q * CH : (q + 1) * CH],
                            s4p[:, q * CH : (q + 1) * CH],
                        )
                    sq_last = sqc

                # ---------------- stage C: final + store (r = it-2) --------
                if it >= 2:
                    r = it - 2
                    sg2r = sig[r % 2]
                    nc.tensor.matmul(
                        OP[0:14, :], cstb[:, CB_WS : CB_WS + 14],
                        sg2r[:, 0:CH],
                        start=False, stop=False, skip_group_check=True,
                        tile_position=(0, 0),
                    )
                    nc.tensor.matmul(
                        OP[32:46, :], cstb[:, CB_WS : CB_WS + 14],
                        sg2r[:, CH : 2 * CH],
                        start=False, stop=True, skip_group_check=True,
                        tile_position=(0, 32),
                    )
                    ou = oup.tile([64, CH], BF16, tag="ou")
                    nc.vector.tensor_copy(ou[0:46, :], OP[0:46, :])
                    nc.sync.dma_start(
                        out=out_d[:, r * CH : (r + 1) * CH], in_=ou[0:46, :]
                    )

                # hoist the last supergroup's u_g pipeline (G matmul, copy,
                # poly) two iterations early so its chain can't gate the drain
                if it == NSG - 2:
                    tl = NSG - 1
                    Gl = ps_g.tile([128, CH], F32, tag="g")
                    nc.tensor.matmul(
                        Gl[:], cstb[:, CB_G : CB_G + 128], xs[tl],
                        start=True, stop=True,
                    )
                    ul = sgp.tile([128, CH], BF16, tag="ug_l")
                    nc.vector.tensor_copy(ul[:], Gl[:])
                    _emit_poly(nc, sgp, sgt, ul, tl, BF16, MULT, ADD,
                               suffix="_l")


    nc.compile()
    _BUILD_CACHE["nc"] = nc
    return nc


def kernel(**inputs):
    inputs = {k: np.asarray(v) for k, v in inputs.items()}
    x = np.ascontiguousarray(inputs["x"], dtype=np.float32)
    assert x.shape == (B, 3 * DOF), x.shape

    fw = fold_weights(inputs)
    cstb = build_const_blobs(fw)

    nc = _build_bass()

    in_maps = []
    for k in range(N_CORES):
        xt = pack_x_core(x[k * BC : (k + 1) * BC])
        in_maps.append({"xt": xt, "cstb": cstb})

    from concourse.bass_utils import run_bass_kernel_spmd

    res = run_bass_kernel_spmd(nc, in_maps, core_ids=list(range(N_CORES)))

    b_out = fw["b_out"]
    out = np.empty((B, DOF), dtype=np.float32)
    for k in range(N_CORES):
        out[k * BC : (k + 1) * BC] = unpack_out_core(res.results[k]["out"], b_out)
    return out
